# revision 2
# baseline (speedup 1.0000x reference)
"""BilinearPooling Trainium2 kernel — saturated-DMA two-ring schedule.

Math: out[b,:] = sign(s_b) * audio[b,:] * rsqrt(sum(audio_b^2)),
s_b = rowsum(visual[b,:])  (algebraically identical to the reference
pooled/normalize; the eps clamp is dead and |s_b| cancels).
visual stays f32 (min_b |s_b| ~ 4e-4, so the sign needs full input
precision); audio/out are bf16 (elementwise relative accuracy only;
measured pipeline rel-err 2.345e-3 vs the 2e-2 gate).

Per-core traffic 16 MiB over 16 shared DMA engines (~410 GB/s aggregate)
-> ~41 us streaming floor; the runtime adds ~5.9 us boot and ~7.8 us
teardown (both fixed, measured via NTFF).

Schedule (8 row-tiles of [128,2048] per core, data parallel on 8 cores):
  Both HWDGE rings issue ALL loads back-to-back up front (an issue
  cascade starves the rings).  Visual and audio interleave so the DVE
  reduce pipeline and the ACT square pipeline both start early; the final
  visual tile per ring is split into column halves so the tail reduce is
  1.2 us, and tail store pairs are split into single tiles so each mul
  releases its store immediately:

    SP  ring: v0 aP0 v1 v4 aP2 v5h1 v5h2 | oP0 o4 o5
    ACT ring: v2 aP1 v3 v6 aP3 v7h1 v7h2 | oP1 o6 o7

  Tail dependencies are ring-aligned (a ring's last stores depend only on
  that ring's loads plus engine time), so a few us of queue-start skew
  shifts a ring's stores together with its slots instead of idling it.

Compute: ACT memzero-free (constants via DVE memset); a dummy
Abs_reciprocal_sqrt is ACT's first activation so the one ACT_TABLE_LOAD
switch happens in the prologue shadow, not mid-pipeline.  ACT: 8 Square+
accum (2.28 us each) + arsq over col quad A and pairs B1/B2.  DVE: 6 full
+ 4 half f32 rowsum reduces, sign chains, 8 bf16 in-place muls (0.75 us).
Small tensors use processing-order columns
COL = {0:0, 2:1, 1:2, 3:3, 4:4, 5:5, 6:6, 7:7}; chain k covers column
pair (2k, 2k+1).  Store issues are gated per-tile on DVE mul marks; each
ring engine waits only its own store receipts.  Entry memsets and
Block-exit drain/barrier are stripped (redundant with the runtime's own
prologue/epilogue sync).
"""

from contextlib import ExitStack

import numpy as np
import ml_dtypes

import concourse.bass as bass
from concourse import mybir
from concourse.bacc import Bacc
from concourse.bass_utils import run_bass_kernel_spmd

B, D = 8192, 2048
N_CORES = 8
ROWS = B // N_CORES          # 1024 rows per core
P = 128
N_TILES = ROWS // P          # 8 row-tiles
H = D // 2                   # column half
FP32 = mybir.dt.float32
BF16 = mybir.dt.bfloat16
INT32 = mybir.dt.int32
AF = mybir.ActivationFunctionType
ALU = mybir.AluOpType

ONE_F32_BITS = 0x3F800000    # bit pattern of float32 1.0

# processing-order column for tile t in the small [P,8] tensors
COL = {0: 0, 2: 1, 1: 2, 3: 3, 4: 4, 5: 5, 6: 6, 7: 7}


class ChainSync:
    """Orders data deps through one per-engine chain semaphore (2-pass)."""

    def __init__(self, sem, preset=None):
        self.sem = sem
        self.count = 0
        self.marks = {}
        self.preset = preset

    def produce(self, inst, key=None):
        if inst is not None:
            inst.then_inc(self.sem, 1)
        self.count += 1
        if key is not None:
            self.marks[key] = self.count
        return inst

    def wait(self, engine, key):
        if self.preset is not None:
            engine.wait_ge(self.sem, self.preset[key])


def build_bass(_marks=None, arsq_func=AF.Abs_reciprocal_sqrt):
    nc = Bacc()
    audio = nc.declare_dram_parameter("audio", [ROWS, D], BF16, isOutput=False)
    visual = nc.declare_dram_parameter("visual", [ROWS, D], FP32, isOutput=False)
    out = nc.declare_dram_parameter("out", [ROWS, D], BF16, isOutput=True)

    # pair j covers rows 256j..256j+255; "(p k) d" puts rows 2p, 2p+1 on
    # partition p -> contiguous 8 KiB DRAM lines for bf16 pairs AND f32
    # tiles.  audio/out/visual all share this row interleave so partition p
    # holds the same row everywhere.
    a_pairs = [
        audio[256 * j : 256 * (j + 1), :].rearrange("(p k) d -> p (k d)", k=2)
        for j in range(4)
    ]
    o_pairs = [
        out[256 * j : 256 * (j + 1), :].rearrange("(p k) d -> p (k d)", k=2)
        for j in range(4)
    ]
    v_pairs = [
        visual[256 * j : 256 * (j + 1), :].rearrange("(p k) d -> p (k d)", k=2)
        for j in range(4)
    ]
    v_tiles_dram = [v_pairs[t // 2][:, (t % 2) * D : (t % 2 + 1) * D] for t in range(N_TILES)]

    with ExitStack() as ctx:
        a_bufs = [
            ctx.enter_context(nc.sbuf_tensor(f"a_buf{j}", [P, 2 * D], BF16))
            for j in range(4)
        ]
        v_bufs = [
            ctx.enter_context(nc.sbuf_tensor(f"v_buf{t}", [P, D], FP32))
            for t in range(N_TILES)
        ]
        scr = [
            ctx.enter_context(nc.sbuf_tensor(f"scr{h}", [P, D], FP32))
            for h in range(2)
        ]
        zero = ctx.enter_context(nc.sbuf_tensor("zero", [P, 1], FP32))
        one_ = ctx.enter_context(nc.sbuf_tensor("one_", [P, 1], FP32))
        q = ctx.enter_context(nc.sbuf_tensor("q", [P, N_TILES], FP32))
        s_ = ctx.enter_context(nc.sbuf_tensor("s_", [P, N_TILES], FP32))
        tmp = ctx.enter_context(nc.sbuf_tensor("tmp", [P, 2], FP32))
        rr = ctx.enter_context(nc.sbuf_tensor("rr", [P, N_TILES], FP32))
        rrd = ctx.enter_context(nc.sbuf_tensor("rrd", [P, 1], FP32))
        sg = ctx.enter_context(nc.sbuf_tensor("sg", [P, N_TILES], FP32))
        sc = ctx.enter_context(nc.sbuf_tensor("sc", [P, N_TILES], FP32))

        # one sem per load transfer
        LS = {
            name: ctx.enter_context(nc.semaphore(name))
            for name in (
                "A0", "A2", "V0", "V1", "V4", "V5A", "V5B",
                "A1", "A3", "V2", "V3", "V6", "V7A", "V7B",
            )
        }
        STS = ctx.enter_context(nc.semaphore("STS"))
        STA = ctx.enter_context(nc.semaphore("STA"))
        DVC = ctx.enter_context(nc.semaphore("DVC"))
        ACC = ctx.enter_context(nc.semaphore("ACC"))

        dv = ChainSync(DVC, preset=None if _marks is None else _marks[0])
        ac = ChainSync(ACC, preset=None if _marks is None else _marks[1])

        def a_tile(t):
            return a_bufs[t // 2][:, (t % 2) * D : (t % 2 + 1) * D]

        block = ctx.enter_context(nc.Block())

        @block.sync
        def _(sp):
            sp.dma_start(out=v_bufs[0][:, :], in_=v_tiles_dram[0]).then_inc(LS["V0"], 16)
            sp.dma_start(out=a_bufs[0][:, :], in_=a_pairs[0]).then_inc(LS["A0"], 16)
            sp.dma_start(out=v_bufs[1][:, :], in_=v_tiles_dram[1]).then_inc(LS["V1"], 16)
            sp.dma_start(out=v_bufs[4][:, :], in_=v_tiles_dram[4]).then_inc(LS["V4"], 16)
            sp.dma_start(out=a_bufs[2][:, :], in_=a_pairs[2]).then_inc(LS["A2"], 16)
            sp.dma_start(out=v_bufs[5][:, 0:H], in_=v_tiles_dram[5][:, 0:H]).then_inc(LS["V5A"], 16)
            sp.dma_start(out=v_bufs[5][:, H:D], in_=v_tiles_dram[5][:, H:D]).then_inc(LS["V5B"], 16)
            dv.wait(sp, ("m", 2))       # mul of tile 1 (tile 0 earlier in DVE order)
            sp.dma_start(out=o_pairs[0], in_=a_bufs[0][:, :]).then_inc(STS, 16)
            dv.wait(sp, ("m", 4))       # mul of tile 4
            sp.dma_start(out=o_pairs[2][:, 0:D], in_=a_bufs[2][:, 0:D]).then_inc(STS, 16)
            dv.wait(sp, ("m", 5))       # mul of tile 5
            sp.dma_start(out=o_pairs[2][:, D : 2 * D], in_=a_bufs[2][:, D : 2 * D]).then_inc(STS, 16)
            sp.wait_ge(STS, 48)

        @block.scalar
        def _(act):
            act.dma_start(out=v_bufs[2][:, :], in_=v_tiles_dram[2]).then_inc(LS["V2"], 16)
            act.dma_start(out=a_bufs[1][:, :], in_=a_pairs[1]).then_inc(LS["A1"], 16)
            act.dma_start(out=v_bufs[3][:, :], in_=v_tiles_dram[3]).then_inc(LS["V3"], 16)
            act.dma_start(out=v_bufs[6][:, :], in_=v_tiles_dram[6]).then_inc(LS["V6"], 16)
            act.dma_start(out=a_bufs[3][:, :], in_=a_pairs[3]).then_inc(LS["A3"], 16)
            act.dma_start(out=v_bufs[7][:, 0:H], in_=v_tiles_dram[7][:, 0:H]).then_inc(LS["V7A"], 16)
            act.dma_start(out=v_bufs[7][:, H:D], in_=v_tiles_dram[7][:, H:D]).then_inc(LS["V7B"], 16)

            # Dummy arsq as the FIRST activation: walrus's ACT_TABLE_LOAD for
            # it picks abs_reciprocal_sqrt_and_small, which also serves
            # Square -> single table load for the whole kernel.
            dv.wait(act, "one")
            ac.produce(
                act.activation(out=rrd[:, :], in_=one_[:, :], func=arsq_func,
                               bias=zero[:, :]),
                "dummy",
            )
            ac.wait(act, "dummy")

            def sq(t, i):
                # scr WAW with the same-parity square two back
                if i >= 2:
                    ac.wait(act, ("sq", i - 2))
                ac.produce(
                    act.activation(
                        out=scr[i % 2][:, :],
                        in_=a_tile(t),
                        func=AF.Square,
                        bias=zero[:, :],
                        accum_out=q[:, COL[t] : COL[t] + 1],
                    ),
                    ("sq", i),
                )

            def arsq(lo, n, k, last_sq):
                # rr[lo:lo+n] = 1/sqrt(|q[lo:lo+n]|)
                ac.wait(act, ("sq", last_sq))
                ac.produce(
                    act.activation(
                        out=rr[:, lo : lo + n],
                        in_=q[:, lo : lo + n],
                        func=arsq_func,
                        bias=zero[:, :],
                    ),
                    ("arsq", k),
                )

            act.wait_ge(LS["A0"], 16)
            sq(0, 0)   # q col 0
            sq(1, 1)   # q col 2
            act.wait_ge(LS["A1"], 16)
            sq(2, 2)   # q col 1
            sq(3, 3)   # q col 3
            arsq(0, 4, 0, 3)            # quad A: cols 0..3
            act.wait_ge(LS["A2"], 16)
            sq(4, 4)   # q col 4
            sq(5, 5)   # q col 5
            arsq(4, 2, 1, 5)            # B1: cols 4,5 (tiles 4,5)
            act.wait_ge(LS["A3"], 16)
            sq(6, 6)   # q col 6
            sq(7, 7)   # q col 7
            arsq(6, 2, 2, 7)            # B2: cols 6,7 (tiles 6,7)

            dv.wait(act, ("m", 3))      # mul of tile 3 (tile 2 earlier)
            act.dma_start(out=o_pairs[1], in_=a_bufs[1][:, :]).then_inc(STA, 16)
            dv.wait(act, ("m", 6))      # mul of tile 6
            act.dma_start(out=o_pairs[3][:, 0:D], in_=a_bufs[3][:, 0:D]).then_inc(STA, 16)
            dv.wait(act, ("m", 7))      # mul of tile 7
            act.dma_start(out=o_pairs[3][:, D : 2 * D], in_=a_bufs[3][:, D : 2 * D]).then_inc(STA, 16)
            act.wait_ge(STA, 48)

        @block.vector
        def _(dve):
            # constants on DVE so ACT's first activation can be the dummy arsq
            dv.produce(dve.memset(zero[:, :], 0.0), "zero")
            dv.wait(dve, "zero")
            dv.produce(dve.memset(one_[:, :], 1.0), "one")

            def reduce_tile(t, sem):
                dve.wait_ge(LS[sem], 16)
                dv.produce(
                    dve.reduce_sum(
                        out=s_[:, COL[t] : COL[t] + 1],
                        in_=v_bufs[t][:, :],
                        axis=mybir.AxisListType.X,
                    ),
                    ("s", t),
                )

            def reduce_half(t, h, semname, col_tmp):
                dve.wait_ge(LS[semname], 16)
                lo = h * H
                dst = s_[:, COL[t] : COL[t] + 1] if h == 0 else tmp[:, col_tmp : col_tmp + 1]
                dv.produce(
                    dve.reduce_sum(
                        out=dst,
                        in_=v_bufs[t][:, lo : lo + H],
                        axis=mybir.AxisListType.X,
                    ),
                    ("sh", (t, h)),
                )

            def add_halves(t, col_tmp):
                dv.wait(dve, ("sh", (t, 0)))
                dv.wait(dve, ("sh", (t, 1)))
                dv.produce(
                    dve.tensor_tensor(
                        out=s_[:, COL[t] : COL[t] + 1],
                        in0=s_[:, COL[t] : COL[t] + 1],
                        in1=tmp[:, col_tmp : col_tmp + 1],
                        op=ALU.add,
                    ),
                    ("s", t),
                )

            def chain(ca, k, arsq_key, ta, tb):
                # sg[ca:ca+2] = 2*(s>=0)-1 ; sc = sg * rr  (cols ca, ca+1)
                dv.wait(dve, ("s", ta))
                dv.wait(dve, ("s", tb))
                dv.produce(
                    dve.tensor_scalar(
                        out=sg[:, ca : ca + 2], in0=s_[:, ca : ca + 2],
                        scalar1=0.0, scalar2=None, op0=ALU.is_ge,
                    ),
                    ("g", k),
                )
                dv.wait(dve, ("g", k))
                dv.produce(
                    dve.tensor_scalar(
                        out=sg[:, ca : ca + 2], in0=sg[:, ca : ca + 2],
                        scalar1=2.0, scalar2=-1.0, op0=ALU.mult, op1=ALU.add,
                    ),
                    ("sg", k),
                )
                dv.wait(dve, ("sg", k))
                ac.wait(dve, ("arsq", arsq_key))
                dv.produce(
                    dve.tensor_tensor(
                        out=sc[:, ca : ca + 2], in0=sg[:, ca : ca + 2],
                        in1=rr[:, ca : ca + 2], op=ALU.mult,
                    ),
                    ("sc", k),
                )

            def mul(t, i, k):
                # in-place a_tile *= sc; ACT's square of tile t is ordered
                # before arsq -> sc -> here, so the WAR on a_tile is covered.
                dv.wait(dve, ("sc", k))
                dv.produce(
                    dve.tensor_scalar_mul(
                        out=a_tile(t), in0=a_tile(t),
                        scalar1=sc[:, COL[t] : COL[t] + 1],
                    ),
                    ("m", i),
                )

            reduce_tile(0, "V0")   # s col 0
            reduce_tile(2, "V2")   # s col 1
            reduce_tile(1, "V1")   # s col 2
            reduce_tile(3, "V3")   # s col 3
            reduce_tile(4, "V4")   # s col 4
            chain(0, 0, 0, 0, 2)
            mul(0, 0, 0)
            mul(2, 1, 0)
            chain(2, 1, 0, 1, 3)
            mul(1, 2, 1)           # -> oP0 (gate ("m",2))
            mul(3, 3, 1)           # -> oP1 (gate ("m",3))
            reduce_tile(6, "V6")   # s col 6
            reduce_half(5, 0, "V5A", 0)
            reduce_half(7, 0, "V7A", 1)
            reduce_half(5, 1, "V5B", 0)
            add_halves(5, 0)
            chain(4, 2, 1, 4, 5)   # cols 4,5 = tiles 4,5; needs arsq B1
            mul(4, 4, 2)           # -> o4 (gate ("m",4))
            mul(5, 5, 2)           # -> o5 (gate ("m",5))
            reduce_half(7, 1, "V7B", 1)
            add_halves(7, 1)
            chain(6, 3, 2, 6, 7)   # cols 6,7 = tiles 6,7; needs arsq B2
            mul(6, 6, 3)           # -> o6 (gate ("m",6))
            mul(7, 7, 3)           # -> o7 (gate ("m",7))

    if _marks is None:
        return build_bass(_marks=(dv.marks, ac.marks), arsq_func=arsq_func)

    # Strip Bass-entry const memsets + barriers and Block-exit drain/barrier
    # (redundant with the runtime's own prologue/epilogue sync; output
    # durability is guaranteed by each ring engine's store-receipt wait).
    for blk in (nc.m.functions[0].blocks[0], nc.m.functions[0].blocks[-1]):
        drop = [
            i
            for i in blk.instructions
            if (
                type(i).__name__ == "InstMemset"
                and any(
                    getattr(o, "memref", "").startswith("const-")
                    for o in (i.outs or [])
                )
            )
            or type(i).__name__ == "InstDrain"
            or i.name.startswith("barrier_")
        ]
        for i in drop:
            blk.instructions.remove(i)

    nc.finalize()
    return nc


_NC = None


def _get_nc():
    global _NC
    if _NC is None:
        _NC = build_bass()
    return _NC


def make_in_maps(audio: np.ndarray, visual: np.ndarray):
    audio = np.ascontiguousarray(audio, dtype=np.float32).astype(ml_dtypes.bfloat16)
    visual = np.ascontiguousarray(visual, dtype=np.float32)
    return [
        {
            "audio": audio[i * ROWS : (i + 1) * ROWS],
            "visual": visual[i * ROWS : (i + 1) * ROWS],
        }
        for i in range(N_CORES)
    ]


def kernel(audio: np.ndarray, visual: np.ndarray) -> np.ndarray:
    nc = _get_nc()
    in_maps = make_in_maps(audio, visual)
    res = run_bass_kernel_spmd(nc, in_maps, core_ids=list(range(N_CORES)))
    return np.concatenate(
        [np.asarray(r["out"]).astype(np.float32) for r in res.results], axis=0
    )


# revision 3
# speedup vs baseline: 1.0226x; 1.0226x over previous
"""BilinearPooling Trainium2 kernel — saturated-DMA two-ring schedule.

Math: out[b,:] = sign(s_b) * audio[b,:] * rsqrt(sum(audio_b^2)),
s_b = rowsum(visual[b,:])  (algebraically identical to the reference
pooled/normalize; the eps clamp is dead and |s_b| cancels).
visual stays f32 (min_b |s_b| ~ 4e-4, the sign needs full input
precision); audio/out are bf16 (measured pipeline rel-err 2.345e-3 vs the
2e-2 gate).

Per-core traffic 16 MiB over 16 shared DMA engines (~410 GB/s aggregate)
-> ~41 us streaming floor; the runtime adds ~5.9 us boot and ~7.8 us
teardown (both fixed, measured via NTFF).

Schedule (8 row-tiles of [128,2048] per core, data parallel on 8 cores):
  Both HWDGE rings issue ALL loads back-to-back up front (an issue
  cascade starves the rings).  Visual and audio interleave so the DVE
  reduce pipeline and the ACT square pipeline both start early; the final
  visual tile per ring is split into column halves (1.2 us tail reduce),
  and tail store pairs are split into single tiles so each mul releases
  its store immediately:

    SP  ring: v0 aP0 v1 v4 aP2 v5h1 v5h2 | oP0 o4 o5
    ACT ring: v2 aP1 v3 v6 aP3 v7h1 v7h2 | oP1 o6 o7

  All tail reduces (r6, v5/v7 halves) are front-loaded on DVE before the
  A-group muls, so the final chain (sign*rsqrt -> mul -> store issue)
  trails the last load by ~3 us and every store is issued before its ring
  slot opens.  Tail dependencies are ring-aligned, so queue-start skew
  shifts a ring's stores together with its slots instead of idling it.

Compute: constants via DVE memset (no ACT memzero); a dummy
Abs_reciprocal_sqrt is ACT's first activation so the activation-table
switch happens in the prologue shadow, not mid-pipeline.  ACT: 8 Square+
accum (2.28 us each) + arsq over col quad A and pairs B1/B2.  DVE: 6 full
+ 4 half f32 rowsum reduces, sign chains, 8 bf16 in-place muls (0.75 us).
Small tensors use processing-order columns
COL = {0:0, 2:1, 1:2, 3:3, 4:4, 5:5, 6:6, 7:7}.  Store issues are gated
per-tile on DVE mul marks; each ring engine waits only its own store
receipts.  Entry memsets and Block-exit drain/barrier are stripped
(redundant with the runtime's own prologue/epilogue sync).
"""

from contextlib import ExitStack

import numpy as np
import ml_dtypes

import concourse.bass as bass
from concourse import mybir
from concourse.bacc import Bacc
from concourse.bass_utils import run_bass_kernel_spmd

B, D = 8192, 2048
N_CORES = 8
ROWS = B // N_CORES          # 1024 rows per core
P = 128
N_TILES = ROWS // P          # 8 row-tiles
H = D // 2                   # column half
FP32 = mybir.dt.float32
BF16 = mybir.dt.bfloat16
INT32 = mybir.dt.int32
AF = mybir.ActivationFunctionType
ALU = mybir.AluOpType

ONE_F32_BITS = 0x3F800000    # bit pattern of float32 1.0

# processing-order column for tile t in the small [P,8] tensors
COL = {0: 0, 2: 1, 1: 2, 3: 3, 4: 4, 5: 5, 6: 6, 7: 7}


class ChainSync:
    """Orders data deps through one per-engine chain semaphore (2-pass)."""

    def __init__(self, sem, preset=None):
        self.sem = sem
        self.count = 0
        self.marks = {}
        self.preset = preset

    def produce(self, inst, key=None):
        if inst is not None:
            inst.then_inc(self.sem, 1)
        self.count += 1
        if key is not None:
            self.marks[key] = self.count
        return inst

    def wait(self, engine, key):
        if self.preset is not None:
            engine.wait_ge(self.sem, self.preset[key])


def build_bass(_marks=None, arsq_func=AF.Abs_reciprocal_sqrt):
    nc = Bacc()
    audio = nc.declare_dram_parameter("audio", [ROWS, D], BF16, isOutput=False)
    visual = nc.declare_dram_parameter("visual", [ROWS, D], FP32, isOutput=False)
    out = nc.declare_dram_parameter("out", [ROWS, D], BF16, isOutput=True)

    # pair j covers rows 256j..256j+255; "(p k) d" puts rows 2p, 2p+1 on
    # partition p -> contiguous 8 KiB DRAM lines for bf16 pairs AND f32
    # tiles.  audio/out/visual all share this row interleave so partition p
    # holds the same row everywhere.
    a_pairs = [
        audio[256 * j : 256 * (j + 1), :].rearrange("(p k) d -> p (k d)", k=2)
        for j in range(4)
    ]
    o_pairs = [
        out[256 * j : 256 * (j + 1), :].rearrange("(p k) d -> p (k d)", k=2)
        for j in range(4)
    ]
    v_pairs = [
        visual[256 * j : 256 * (j + 1), :].rearrange("(p k) d -> p (k d)", k=2)
        for j in range(4)
    ]
    v_tiles_dram = [v_pairs[t // 2][:, (t % 2) * D : (t % 2 + 1) * D] for t in range(N_TILES)]

    with ExitStack() as ctx:
        a_bufs = [
            ctx.enter_context(nc.sbuf_tensor(f"a_buf{j}", [P, 2 * D], BF16))
            for j in range(4)
        ]
        v_bufs = [
            ctx.enter_context(nc.sbuf_tensor(f"v_buf{t}", [P, D], FP32))
            for t in range(N_TILES)
        ]
        scr = [
            ctx.enter_context(nc.sbuf_tensor(f"scr{h}", [P, D], FP32))
            for h in range(2)
        ]
        zero = ctx.enter_context(nc.sbuf_tensor("zero", [P, 1], FP32))
        one_ = ctx.enter_context(nc.sbuf_tensor("one_", [P, 1], FP32))
        q = ctx.enter_context(nc.sbuf_tensor("q", [P, N_TILES], FP32))
        s_ = ctx.enter_context(nc.sbuf_tensor("s_", [P, N_TILES], FP32))
        tmp = ctx.enter_context(nc.sbuf_tensor("tmp", [P, 2], FP32))
        rr = ctx.enter_context(nc.sbuf_tensor("rr", [P, N_TILES], FP32))
        rrd = ctx.enter_context(nc.sbuf_tensor("rrd", [P, 1], FP32))
        sg = ctx.enter_context(nc.sbuf_tensor("sg", [P, N_TILES], FP32))
        sc = ctx.enter_context(nc.sbuf_tensor("sc", [P, N_TILES], FP32))

        # one sem per load transfer
        LS = {
            name: ctx.enter_context(nc.semaphore(name))
            for name in (
                "A0", "A2", "V0", "V1", "V4", "V5A", "V5B",
                "A1", "A3", "V2", "V3", "V6", "V7A", "V7B",
            )
        }
        STS = ctx.enter_context(nc.semaphore("STS"))
        STA = ctx.enter_context(nc.semaphore("STA"))
        DVC = ctx.enter_context(nc.semaphore("DVC"))
        ACC = ctx.enter_context(nc.semaphore("ACC"))

        dv = ChainSync(DVC, preset=None if _marks is None else _marks[0])
        ac = ChainSync(ACC, preset=None if _marks is None else _marks[1])

        def a_tile(t):
            return a_bufs[t // 2][:, (t % 2) * D : (t % 2 + 1) * D]

        block = ctx.enter_context(nc.Block())

        @block.sync
        def _(sp):
            sp.dma_start(out=v_bufs[0][:, :], in_=v_tiles_dram[0]).then_inc(LS["V0"], 16)
            sp.dma_start(out=a_bufs[0][:, :], in_=a_pairs[0]).then_inc(LS["A0"], 16)
            sp.dma_start(out=v_bufs[1][:, :], in_=v_tiles_dram[1]).then_inc(LS["V1"], 16)
            sp.dma_start(out=v_bufs[4][:, :], in_=v_tiles_dram[4]).then_inc(LS["V4"], 16)
            sp.dma_start(out=a_bufs[2][:, :], in_=a_pairs[2]).then_inc(LS["A2"], 16)
            sp.dma_start(out=v_bufs[5][:, 0:H], in_=v_tiles_dram[5][:, 0:H]).then_inc(LS["V5A"], 16)
            sp.dma_start(out=v_bufs[5][:, H:D], in_=v_tiles_dram[5][:, H:D]).then_inc(LS["V5B"], 16)
            dv.wait(sp, ("m", 2))       # mul of tile 1 (tile 0 earlier in DVE order)
            sp.dma_start(out=o_pairs[0], in_=a_bufs[0][:, :]).then_inc(STS, 16)
            dv.wait(sp, ("m", 4))       # mul of tile 4
            sp.dma_start(out=o_pairs[2][:, 0:D], in_=a_bufs[2][:, 0:D]).then_inc(STS, 16)
            dv.wait(sp, ("m", 5))       # mul of tile 5
            sp.dma_start(out=o_pairs[2][:, D : 2 * D], in_=a_bufs[2][:, D : 2 * D]).then_inc(STS, 16)
            sp.wait_ge(STS, 48)

        @block.scalar
        def _(act):
            act.dma_start(out=v_bufs[2][:, :], in_=v_tiles_dram[2]).then_inc(LS["V2"], 16)
            act.dma_start(out=a_bufs[1][:, :], in_=a_pairs[1]).then_inc(LS["A1"], 16)
            act.dma_start(out=v_bufs[3][:, :], in_=v_tiles_dram[3]).then_inc(LS["V3"], 16)
            act.dma_start(out=v_bufs[6][:, :], in_=v_tiles_dram[6]).then_inc(LS["V6"], 16)
            act.dma_start(out=a_bufs[3][:, :], in_=a_pairs[3]).then_inc(LS["A3"], 16)
            act.dma_start(out=v_bufs[7][:, 0:H], in_=v_tiles_dram[7][:, 0:H]).then_inc(LS["V7A"], 16)
            act.dma_start(out=v_bufs[7][:, H:D], in_=v_tiles_dram[7][:, H:D]).then_inc(LS["V7B"], 16)

            # Dummy arsq as the FIRST activation: walrus's ACT_TABLE_LOAD for
            # it picks abs_reciprocal_sqrt_and_small, which also serves
            # Square -> single table load for the whole kernel.
            dv.wait(act, "one")
            ac.produce(
                act.activation(out=rrd[:, :], in_=one_[:, :], func=arsq_func,
                               bias=zero[:, :]),
                "dummy",
            )
            ac.wait(act, "dummy")

            def sq(t, i):
                # scr WAW with the same-parity square two back
                if i >= 2:
                    ac.wait(act, ("sq", i - 2))
                ac.produce(
                    act.activation(
                        out=scr[i % 2][:, :],
                        in_=a_tile(t),
                        func=AF.Square,
                        bias=zero[:, :],
                        accum_out=q[:, COL[t] : COL[t] + 1],
                    ),
                    ("sq", i),
                )

            def arsq(lo, n, k, last_sq):
                # rr[lo:lo+n] = 1/sqrt(|q[lo:lo+n]|)
                ac.wait(act, ("sq", last_sq))
                ac.produce(
                    act.activation(
                        out=rr[:, lo : lo + n],
                        in_=q[:, lo : lo + n],
                        func=arsq_func,
                        bias=zero[:, :],
                    ),
                    ("arsq", k),
                )

            act.wait_ge(LS["A0"], 16)
            sq(0, 0)   # q col 0
            sq(1, 1)   # q col 2
            act.wait_ge(LS["A1"], 16)
            sq(2, 2)   # q col 1
            sq(3, 3)   # q col 3
            arsq(0, 4, 0, 3)            # quad A: cols 0..3
            act.wait_ge(LS["A2"], 16)
            sq(4, 4)   # q col 4
            sq(5, 5)   # q col 5
            arsq(4, 2, 1, 5)            # B1: cols 4,5 (tiles 4,5)
            act.wait_ge(LS["A3"], 16)
            sq(6, 6)   # q col 6
            sq(7, 7)   # q col 7
            arsq(6, 2, 2, 7)            # B2: cols 6,7 (tiles 6,7)

            dv.wait(act, ("m", 3))      # mul of tile 3 (tile 2 earlier)
            act.dma_start(out=o_pairs[1], in_=a_bufs[1][:, :]).then_inc(STA, 16)
            dv.wait(act, ("m", 6))      # mul of tile 6
            act.dma_start(out=o_pairs[3][:, 0:D], in_=a_bufs[3][:, 0:D]).then_inc(STA, 16)
            dv.wait(act, ("m", 7))      # mul of tile 7
            act.dma_start(out=o_pairs[3][:, D : 2 * D], in_=a_bufs[3][:, D : 2 * D]).then_inc(STA, 16)
            act.wait_ge(STA, 48)

        @block.vector
        def _(dve):
            # constants on DVE so ACT's first activation can be the dummy arsq
            dv.produce(dve.memset(zero[:, :], 0.0), "zero")
            dv.wait(dve, "zero")
            dv.produce(dve.memset(one_[:, :], 1.0), "one")

            def reduce_tile(t, sem):
                dve.wait_ge(LS[sem], 16)
                dv.produce(
                    dve.reduce_sum(
                        out=s_[:, COL[t] : COL[t] + 1],
                        in_=v_bufs[t][:, :],
                        axis=mybir.AxisListType.X,
                    ),
                    ("s", t),
                )

            def reduce_half(t, h, semname, col_tmp):
                dve.wait_ge(LS[semname], 16)
                lo = h * H
                dst = s_[:, COL[t] : COL[t] + 1] if h == 0 else tmp[:, col_tmp : col_tmp + 1]
                dv.produce(
                    dve.reduce_sum(
                        out=dst,
                        in_=v_bufs[t][:, lo : lo + H],
                        axis=mybir.AxisListType.X,
                    ),
                    ("sh", (t, h)),
                )

            def add_halves(t, col_tmp):
                dv.wait(dve, ("sh", (t, 0)))
                dv.wait(dve, ("sh", (t, 1)))
                dv.produce(
                    dve.tensor_tensor(
                        out=s_[:, COL[t] : COL[t] + 1],
                        in0=s_[:, COL[t] : COL[t] + 1],
                        in1=tmp[:, col_tmp : col_tmp + 1],
                        op=ALU.add,
                    ),
                    ("s", t),
                )

            def chain(ca, k, arsq_key, ta, tb):
                # sg[ca:ca+2] = 2*(s>=0)-1 ; sc = sg * rr  (cols ca, ca+1)
                dv.wait(dve, ("s", ta))
                dv.wait(dve, ("s", tb))
                dv.produce(
                    dve.tensor_scalar(
                        out=sg[:, ca : ca + 2], in0=s_[:, ca : ca + 2],
                        scalar1=0.0, scalar2=None, op0=ALU.is_ge,
                    ),
                    ("g", k),
                )
                dv.wait(dve, ("g", k))
                dv.produce(
                    dve.tensor_scalar(
                        out=sg[:, ca : ca + 2], in0=sg[:, ca : ca + 2],
                        scalar1=2.0, scalar2=-1.0, op0=ALU.mult, op1=ALU.add,
                    ),
                    ("sg", k),
                )
                dv.wait(dve, ("sg", k))
                ac.wait(dve, ("arsq", arsq_key))
                dv.produce(
                    dve.tensor_tensor(
                        out=sc[:, ca : ca + 2], in0=sg[:, ca : ca + 2],
                        in1=rr[:, ca : ca + 2], op=ALU.mult,
                    ),
                    ("sc", k),
                )

            def mul(t, i, k):
                # in-place a_tile *= sc; ACT's square of tile t is ordered
                # before arsq -> sc -> here, so the WAR on a_tile is covered.
                dv.wait(dve, ("sc", k))
                dv.produce(
                    dve.tensor_scalar_mul(
                        out=a_tile(t), in0=a_tile(t),
                        scalar1=sc[:, COL[t] : COL[t] + 1],
                    ),
                    ("m", i),
                )

            reduce_tile(0, "V0")   # s col 0
            reduce_tile(2, "V2")   # s col 1
            reduce_tile(1, "V1")   # s col 2
            reduce_tile(3, "V3")   # s col 3
            reduce_tile(4, "V4")   # s col 4
            chain(0, 0, 0, 0, 2)
            chain(2, 1, 0, 1, 3)
            reduce_tile(6, "V6")   # s col 6 (early: tail chain depends on it)
            mul(0, 0, 0)
            mul(2, 1, 0)
            mul(1, 2, 1)           # -> oP0 (gate ("m",2))
            mul(3, 3, 1)           # -> oP1 (gate ("m",3))
            reduce_half(5, 0, "V5A", 0)
            reduce_half(7, 0, "V7A", 1)
            reduce_half(5, 1, "V5B", 0)
            add_halves(5, 0)
            reduce_half(7, 1, "V7B", 1)
            add_halves(7, 1)
            chain(4, 2, 1, 4, 5)   # cols 4,5 = tiles 4,5; needs arsq B1
            mul(4, 4, 2)           # -> o4 (gate ("m",4))
            mul(5, 5, 2)           # -> o5 (gate ("m",5))
            chain(6, 3, 2, 6, 7)   # cols 6,7 = tiles 6,7; needs arsq B2
            mul(6, 6, 3)           # -> o6 (gate ("m",6))
            mul(7, 7, 3)           # -> o7 (gate ("m",7))

    if _marks is None:
        return build_bass(_marks=(dv.marks, ac.marks), arsq_func=arsq_func)

    # Strip Bass-entry const memsets + barriers and Block-exit drain/barrier
    # (redundant with the runtime's own prologue/epilogue sync; output
    # durability is guaranteed by each ring engine's store-receipt wait).
    for blk in (nc.m.functions[0].blocks[0], nc.m.functions[0].blocks[-1]):
        drop = [
            i
            for i in blk.instructions
            if (
                type(i).__name__ == "InstMemset"
                and any(
                    getattr(o, "memref", "").startswith("const-")
                    for o in (i.outs or [])
                )
            )
            or type(i).__name__ == "InstDrain"
            or i.name.startswith("barrier_")
        ]
        for i in drop:
            blk.instructions.remove(i)

    nc.finalize()
    return nc


_NC = None


def _get_nc():
    global _NC
    if _NC is None:
        _NC = build_bass()
    return _NC


def make_in_maps(audio: np.ndarray, visual: np.ndarray):
    audio = np.ascontiguousarray(audio, dtype=np.float32).astype(ml_dtypes.bfloat16)
    visual = np.ascontiguousarray(visual, dtype=np.float32)
    return [
        {
            "audio": audio[i * ROWS : (i + 1) * ROWS],
            "visual": visual[i * ROWS : (i + 1) * ROWS],
        }
        for i in range(N_CORES)
    ]


def kernel(audio: np.ndarray, visual: np.ndarray) -> np.ndarray:
    nc = _get_nc()
    in_maps = make_in_maps(audio, visual)
    res = run_bass_kernel_spmd(nc, in_maps, core_ids=list(range(N_CORES)))
    return np.concatenate(
        [np.asarray(r["out"]).astype(np.float32) for r in res.results], axis=0
    )
